# revision 8
# baseline (speedup 1.0000x reference)
"""Causal self-attention on 8 NeuronCores (Trainium2, Bass/Tile).

Sharding: core c handles batch b = c//2 and head-group hg = c%2
(8 of 16 heads = 512 of 1024 feature dims). W_qkv is split column-wise,
W_proj row-wise per head group; each core returns a partial [T, D]
projection output and the host sums the two partials per batch.

Per-core dataflow (all matmuls fp32r except P/V which are bf16):
  xT = embds[b].T              [1024, 2048]  (host-transposed)
  qT/kT = Wq/Wk.T @ x.T        [512, 2048]   (head-dim major)
  v     = x @ Wv               [2048, 512]   (natural, + ones col per head)
  sT[j,i] = kT.T @ qT          per head, causal-skipped/shrunk tiles
  PT = exp(SCALE * sT) (*mask on diagonal strips)   bf16
  UT[e,i], denom[i] = [v|1].T @ PT                  (ones col -> denom)
  affinT = UT * (1/denom)      broadcast via K=8 matmul with E matrix
  partial = affinT.T @ Wp      accumulated over e-chunks, DMA'd out
"""

import sys

for _p in ("/opt/trn_rl_repo",):
    if _p not in sys.path:
        sys.path.append(_p)

import ml_dtypes
import numpy as np

import concourse.bass as bass
import concourse.tile as tile
from concourse import bacc, mybir
from concourse.bass_utils import run_bass_kernel_spmd

F32 = mybir.dt.float32
F32R = mybir.dt.float32r
BF16 = mybir.dt.bfloat16
EXP = mybir.ActivationFunctionType.Exp
COPY = mybir.ActivationFunctionType.Copy

B, T, D = 4, 2048, 1024
H, Dh = 16, 64
SCALE = float(D) ** -0.5
NCORES = 8
DL = 512          # local (per-core) feature width = 8 heads * 64
HL = 8            # local heads
NDC = D // 128    # 8 d-chunks
NEC = DL // 128   # 4 e-chunks (head pairs)
NTB = T // 512    # 4 t-blocks of 512
NTC = T // 128    # 16 t-chunks of 128
VPAIR = 192       # v_sb per-pair block: [v_even(64) | one | junk(63) | v_odd(64)]
VROW = NEC * VPAIR  # 640 cols per v_sb tile


def _build():
    nc = bacc.Bacc("TRN2", target_bir_lowering=False, debug=False,
                   num_devices=NCORES)

    xT = nc.declare_dram_parameter("xT", [D, T], F32R, isOutput=False)
    wq = nc.declare_dram_parameter("wq", [D, DL], F32R, isOutput=False)
    wk = nc.declare_dram_parameter("wk", [D, DL], F32R, isOutput=False)
    wv = nc.declare_dram_parameter("wv", [D, DL], F32R, isOutput=False)
    wp = nc.declare_dram_parameter("wp", [DL, D], F32R, isOutput=False)
    mask = nc.declare_dram_parameter("mask", [128, 128], BF16, isOutput=False)
    emat = nc.declare_dram_parameter("emat", [HL, DL], F32R, isOutput=False)
    out = nc.declare_dram_parameter("out", [T, D], F32, isOutput=True)

    with tile.TileContext(nc) as tc:
        _emit(nc, tc, xT, wq, wk, wv, wp, mask, emat, out)
    nc.compile()
    return nc


def _emit(nc, tc, xT, wq, wk, wv, wp, mask, emat, out):
    from contextlib import ExitStack

    ctx = ExitStack()
    with ctx:
        wqk_pool = ctx.enter_context(tc.tile_pool(name="wqk", bufs=24))
        qk_pool = ctx.enter_context(tc.tile_pool(name="qk", bufs=3))
        vsb_pool = ctx.enter_context(tc.tile_pool(name="vsb", bufs=NTC))
        ut_pool = ctx.enter_context(tc.tile_pool(name="ut", bufs=NEC))
        dn_pool = ctx.enter_context(tc.tile_pool(name="dn", bufs=1))
        dns_pool = ctx.enter_context(tc.tile_pool(name="dns", bufs=2))
        pt_pool = ctx.enter_context(tc.tile_pool(name="pt", bufs=3))
        cst_pool = ctx.enter_context(tc.tile_pool(name="cst", bufs=1))
        ps_pool = ctx.enter_context(tc.tile_pool(name="ps", bufs=1, space="PSUM"))

        # constants
        mk_sb = cst_pool.tile([128, 128], BF16, tag="mk")
        nc.sync.dma_start(mk_sb[:], mask[:])
        em_sb = cst_pool.tile([HL, DL], F32R, tag="em")
        nc.sync.dma_start(em_sb[:], emat[:])

        # persistent outputs of phase A/B
        ut_sb = [ut_pool.tile([128, T], F32R, tag="ut", name=f"ut{i}") for i in range(NEC)]
        dn_sb = dn_pool.tile([HL, T], F32, tag="dn")
        v_sb = [vsb_pool.tile([128, VROW], BF16, tag="vsb", name=f"vsb{i}") for i in range(NTC)]

        def ps_tile(tag, bufs):
            return ps_pool.tile([128, 512], F32, tag=tag, name=f"ps_{tag}",
                                bufs=bufs)

        # ---------------- phase A0: load xT / wv; compute v ----------------
        with tc.tile_pool(name="xt", bufs=NDC) as xt_pool, \
             tc.tile_pool(name="wv", bufs=NDC) as wv_pool:
            xt_sb = []
            for dc in range(NDC):
                t = xt_pool.tile([128, T], F32R, tag="xt", name=f"xt{dc}")
                nc.sync.dma_start(t[:], xT[dc * 128:(dc + 1) * 128, :])
                xt_sb.append(t)
            wv_sb = []
            for dc in range(NDC):
                t = wv_pool.tile([128, DL], F32R, tag="wv", name=f"wv{dc}")
                nc.sync.dma_start(t[:], wv[dc * 128:(dc + 1) * 128, :])
                wv_sb.append(t)

            # v: per t-chunk, [t(128), vcol(512)] = xT_chunk.T @ wv
            for tcn in range(NTC):
                ps_v = ps_tile("qkps", 2)
                for dc in range(NDC):
                    nc.tensor.matmul(
                        ps_v[:], xt_sb[dc][:, tcn * 128:(tcn + 1) * 128],
                        wv_sb[dc][:], start=(dc == 0), stop=(dc == NDC - 1))
                # scatter into v_sb pair-blocks (bf16 cast on copy)
                dst = v_sb[tcn][:].rearrange("p (e c) -> p e c", c=VPAIR)
                src = ps_v[:].rearrange("p (e c) -> p e c", c=128)
                nc.vector.tensor_copy(dst[:, :, 0:64], src[:, :, 0:64])
                nc.vector.tensor_copy(dst[:, :, 128:192], src[:, :, 64:128])
                nc.gpsimd.memset(dst[:, :, 64:65], 1.0)
                nc.gpsimd.memset(dst[:, :, 65:128], 0.0)

            # ------------- phase A1..A4: per e-chunk qk + attention -------------
            for ec in range(NEC):
                # load weight slices for this e-chunk
                wq_t, wk_t = [], []
                for dc in range(NDC):
                    t = wqk_pool.tile([128, 128], F32R, tag="wqk", name="wqkt")
                    nc.sync.dma_start(
                        t[:], wq[dc * 128:(dc + 1) * 128, ec * 128:(ec + 1) * 128])
                    wq_t.append(t)
                for dc in range(NDC):
                    t = wqk_pool.tile([128, 128], F32R, tag="wqk", name="wqkt")
                    nc.sync.dma_start(
                        t[:], wk[dc * 128:(dc + 1) * 128, ec * 128:(ec + 1) * 128])
                    wk_t.append(t)

                # qT / kT chunk [128, T] = W_chunk.T @ xT
                q_ec = qk_pool.tile([128, T], F32R, tag="qk")
                k_ec = qk_pool.tile([128, T], F32R, tag="qk")
                for (w_t, dst_t) in ((wq_t, q_ec), (wk_t, k_ec)):
                    for tbp in range(2):  # t-block pairs, 2 psum tiles each
                        pss = [ps_tile("qkps", 2) for _ in range(2)]
                        for dc in range(NDC):
                            for i in range(2):
                                tb = 2 * tbp + i
                                nc.tensor.matmul(
                                    pss[i], w_t[dc][:],
                                    xt_sb[dc][:, tb * 512:(tb + 1) * 512],
                                    start=(dc == 0), stop=(dc == NDC - 1))
                        for i in range(2):
                            tb = 2 * tbp + i
                            nc.vector.tensor_copy(
                                dst_t[:, tb * 512:(tb + 1) * 512], pss[i][:])

                # attention for the two heads of this chunk
                for par in range(2):       # head parity within chunk
                    h = 2 * ec + par       # local head index
                    po = 64 * par          # partition offset in qk chunk
                    qh = q_ec[po:po + 64, :]
                    kh = k_ec[po:po + 64, :]
                    ut_ps = [ps_tile("utps", 4) for _ in range(4)]
                    for jt in range(NTC):
                        ib0 = jt // 4
                        for ib in range(ib0, 4):
                            diag = (ib == ib0)
                            c0 = 128 * (jt - 4 * ib) if diag else 0
                            st_ps = ps_tile("stps", 2)
                            nc.tensor.matmul(
                                st_ps[:, c0:512],
                                kh[:, jt * 128:(jt + 1) * 128],
                                qh[:, ib * 512 + c0:(ib + 1) * 512],
                                start=True, stop=True)
                            pt_t = pt_pool.tile([128, 512], BF16, tag="pt")
                            nc.scalar.activation(
                                pt_t[:, c0:512], st_ps[:, c0:512], EXP,
                                scale=SCALE)
                            if diag:
                                nc.vector.tensor_mul(
                                    pt_t[:, c0:c0 + 128], pt_t[:, c0:c0 + 128],
                                    mk_sb[:])
                            # PV: [v|1].T @ PT -> UT rows + denom row
                            vt = v_sb[jt][:].rearrange(
                                "p (e c) -> p e c", c=VPAIR)[:, ec, :]
                            if par == 0:
                                lhs = vt[:, 0:65]          # M=65 -> rows 0..64
                                uslc = ut_ps[ib][0:65, c0:512]
                            else:
                                # [one|junk63|v_odd]: denom -> row 0, v -> 64..127
                                lhs = vt[:, 64:192]        # M=128 -> rows 0..127
                                uslc = ut_ps[ib][0:128, c0:512]
                            nc.tensor.matmul(
                                uslc, lhs, pt_t[:, c0:512],
                                start=(jt == 0), stop=(jt == 4 * ib + 3),
                                skip_group_check=True)
                        if jt % 4 == 3:
                            ib_d = jt // 4
                            if par == 0:
                                usrc, dsrc, r = (ut_ps[ib_d][0:64, :],
                                                 ut_ps[ib_d][64:65, :], 64)
                                udst = ut_sb[ec][0:64, ib_d * 512:(ib_d + 1) * 512]
                            else:
                                usrc, dsrc, r = (ut_ps[ib_d][64:128, :],
                                                 ut_ps[ib_d][0:1, :], 0)
                                udst = ut_sb[ec][64:128, ib_d * 512:(ib_d + 1) * 512]
                            nc.vector.tensor_copy(udst, usrc)
                            # denom: same-partition copy to stage, DMA repack
                            stg = dns_pool.tile([128, 512], F32, tag="dns",
                                                name="dnstg")
                            nc.vector.tensor_copy(stg[r:r + 1, :], dsrc)
                            nc.sync.dma_start(
                                dn_sb[h:h + 1, ib_d * 512:(ib_d + 1) * 512],
                                stg[r:r + 1, :])
        # xt/wv pools released here

        # ---------------- normalize: affinT = UT * (1/denom) ----------------
        rd = qk_pool.tile([HL, T], F32R, tag="qk")
        tmp = qk_pool.tile([HL, T], F32, tag="qk")
        r0 = qk_pool.tile([HL, T], F32, tag="qk")
        nc.vector.reciprocal(r0[:], dn_sb[:])
        # one Newton step: rd = r0 * (2 - dn*r0)
        nc.vector.tensor_mul(tmp[:], dn_sb[:], r0[:])
        nc.scalar.activation(tmp[:], tmp[:], COPY, scale=-1.0, bias=2.0)
        nc.vector.tensor_mul(rd[:], r0[:], tmp[:])

        for ec in range(NEC):
            for tb in range(NTB):
                ps_r = ps_tile("qkps", 2)
                nc.tensor.matmul(
                    ps_r[:], em_sb[:, ec * 128:(ec + 1) * 128],
                    rd[:, tb * 512:(tb + 1) * 512], start=True, stop=True)
                nc.vector.tensor_mul(
                    ut_sb[ec][:, tb * 512:(tb + 1) * 512],
                    ut_sb[ec][:, tb * 512:(tb + 1) * 512], ps_r[:])

        # ---------------- projection: out = affinT.T @ wp ----------------
        with tc.tile_pool(name="wp", bufs=NEC) as wp_pool, \
             tc.tile_pool(name="stage", bufs=3) as stage_pool:
            wp_sb = []
            for ecn in range(NEC):
                t = wp_pool.tile([128, D], F32R, tag="wp", name=f"wpt{ecn}")
                nc.sync.dma_start(t[:], wp[ecn * 128:(ecn + 1) * 128, :])
                wp_sb.append(t)
            for tcn in range(NTC):
                for ob in range(2):
                    ps_p = ps_tile("qkps", 2)
                    for ecn in range(NEC):
                        nc.tensor.matmul(
                            ps_p[:], ut_sb[ecn][:, tcn * 128:(tcn + 1) * 128],
                            wp_sb[ecn][:, ob * 512:(ob + 1) * 512],
                            start=(ecn == 0), stop=(ecn == NEC - 1))
                    st = stage_pool.tile([128, 512], F32, tag="st")
                    nc.vector.tensor_copy(st[:], ps_p[:])
                    nc.sync.dma_start(
                        out[tcn * 128:(tcn + 1) * 128, ob * 512:(ob + 1) * 512],
                        st[:])


_NC_CACHE = None


def _get_nc():
    global _NC_CACHE
    if _NC_CACHE is None:
        _NC_CACHE = _build()
    return _NC_CACHE


def make_in_maps(embds, W_qkv, W_proj):
    embds = np.asarray(embds, dtype=np.float32)
    W_qkv = np.asarray(W_qkv, dtype=np.float32)
    W_proj = np.asarray(W_proj, dtype=np.float32)

    mask_np = np.triu(np.ones((128, 128))).astype(ml_dtypes.bfloat16)
    emat_np = np.kron(np.eye(HL), np.ones((1, Dh))).astype(np.float32)

    in_maps = []
    for c in range(NCORES):
        b, hg = c // 2, c % 2
        sl = slice(hg * DL, (hg + 1) * DL)
        in_maps.append({
            "xT": np.ascontiguousarray(embds[b].T),
            "wk": np.ascontiguousarray(W_qkv[:, 0 * D:1 * D][:, sl]),
            "wq": np.ascontiguousarray(W_qkv[:, 1 * D:2 * D][:, sl]),
            "wv": np.ascontiguousarray(W_qkv[:, 2 * D:3 * D][:, sl]),
            "wp": np.ascontiguousarray(W_proj[sl, :]),
            "mask": mask_np,
            "emat": emat_np,
        })
    return in_maps


def gather_out(outs, b_proj):
    b_proj = np.asarray(b_proj, dtype=np.float32)
    full = np.empty((B, T, D), dtype=np.float32)
    for b in range(B):
        full[b] = outs[2 * b] + outs[2 * b + 1] + b_proj[None, :]
    return full


def kernel(embds, W_qkv, W_proj, b_proj):
    in_maps = make_in_maps(embds, W_qkv, W_proj)
    nc = _get_nc()
    res = run_bass_kernel_spmd(nc, in_maps, list(range(NCORES)))
    return gather_out([r["out"] for r in res.results], b_proj)


# revision 9
# speedup vs baseline: 1.1265x; 1.1265x over previous
"""Causal self-attention on 8 NeuronCores (Trainium2, Bass/Tile).

Sharding: core c handles batch b = c//2 and head-group hg = c%2
(8 of 16 heads = 512 of 1024 feature dims). W_qkv is split column-wise,
W_proj row-wise per head group; each core returns a partial [T, D]
projection output and the host sums the two partials per batch.

Per-core dataflow (all matmuls fp32r except P/V which are bf16):
  xT = embds[b].T              [1024, 2048]  (host-transposed)
  qT/kT = Wq/Wk.T @ x.T        [512, 2048]   (head-dim major)
  v     = x @ Wv               [2048, 512]   (natural, + ones col per head)
  sT[j,i] = kT.T @ qT          per head, causal-skipped/shrunk tiles
  PT = exp(SCALE * sT) (*mask on diagonal strips)   bf16
  UT[e,i], denom[i] = [v|1].T @ PT                  (ones col -> denom)
  affinT = UT * (1/denom)      broadcast via K=8 matmul with E matrix
  partial = affinT.T @ Wp      accumulated over e-chunks, DMA'd out
"""

import sys

for _p in ("/opt/trn_rl_repo",):
    if _p not in sys.path:
        sys.path.append(_p)

import ml_dtypes
import numpy as np

import concourse.bass as bass
import concourse.tile as tile
from concourse import bacc, mybir
from concourse.bass_utils import run_bass_kernel_spmd

F32 = mybir.dt.float32
F32R = mybir.dt.float32r
BF16 = mybir.dt.bfloat16
EXP = mybir.ActivationFunctionType.Exp
COPY = mybir.ActivationFunctionType.Copy

B, T, D = 4, 2048, 1024
H, Dh = 16, 64
SCALE = float(D) ** -0.5
NCORES = 8
DL = 512          # local (per-core) feature width = 8 heads * 64
HL = 8            # local heads
NDC = D // 128    # 8 d-chunks
NEC = DL // 128   # 4 e-chunks (head pairs)
NTB = T // 512    # 4 t-blocks of 512
NTC = T // 128    # 16 t-chunks of 128
VPAIR = 192       # v_sb per-pair block: [v_even(64) | one | junk(63) | v_odd(64)]
VROW = NEC * VPAIR  # 640 cols per v_sb tile


def _build():
    nc = bacc.Bacc("TRN2", target_bir_lowering=False, debug=False,
                   num_devices=NCORES)

    xT = nc.declare_dram_parameter("xT", [D, T], F32R, isOutput=False)
    wq = nc.declare_dram_parameter("wq", [D, DL], F32R, isOutput=False)
    wk = nc.declare_dram_parameter("wk", [D, DL], F32R, isOutput=False)
    wv = nc.declare_dram_parameter("wv", [D, DL], F32R, isOutput=False)
    wp = nc.declare_dram_parameter("wp", [DL, D], F32R, isOutput=False)
    mask = nc.declare_dram_parameter("mask", [128, 128], BF16, isOutput=False)
    emat = nc.declare_dram_parameter("emat", [HL, DL], F32R, isOutput=False)
    out = nc.declare_dram_parameter("out", [T, D], F32, isOutput=True)

    with tile.TileContext(nc) as tc:
        _emit(nc, tc, xT, wq, wk, wv, wp, mask, emat, out)
    nc.compile()
    return nc


def _emit(nc, tc, xT, wq, wk, wv, wp, mask, emat, out):
    from contextlib import ExitStack

    ctx = ExitStack()
    with ctx:
        wqk_pool = ctx.enter_context(tc.tile_pool(name="wqk", bufs=24))
        qk_pool = ctx.enter_context(tc.tile_pool(name="qk", bufs=3))
        vsb_pool = ctx.enter_context(tc.tile_pool(name="vsb", bufs=NTC))
        ut_pool = ctx.enter_context(tc.tile_pool(name="ut", bufs=NEC))
        dn_pool = ctx.enter_context(tc.tile_pool(name="dn", bufs=1))
        dns_pool = ctx.enter_context(tc.tile_pool(name="dns", bufs=2))
        pt_pool = ctx.enter_context(tc.tile_pool(name="pt", bufs=3))
        cst_pool = ctx.enter_context(tc.tile_pool(name="cst", bufs=1))
        ps_pool = ctx.enter_context(tc.tile_pool(name="ps", bufs=1, space="PSUM"))

        # constants
        mk_sb = cst_pool.tile([128, 128], BF16, tag="mk")
        nc.sync.dma_start(mk_sb[:], mask[:])
        em_sb = cst_pool.tile([HL, DL], F32R, tag="em")
        nc.sync.dma_start(em_sb[:], emat[:])

        # persistent outputs of phase A/B
        ut_sb = [ut_pool.tile([128, T], F32R, tag="ut", name=f"ut{i}") for i in range(NEC)]
        dn_sb = dn_pool.tile([HL, T], F32, tag="dn")
        v_sb = [vsb_pool.tile([128, VROW], BF16, tag="vsb", name=f"vsb{i}") for i in range(NTC)]

        def ps_tile(tag, bufs):
            return ps_pool.tile([128, 512], F32, tag=tag, name=f"ps_{tag}",
                                bufs=bufs)

        # ---------------- phase A0: load xT / wv; compute v ----------------
        with tc.tile_pool(name="xt", bufs=NDC) as xt_pool, \
             tc.tile_pool(name="wv", bufs=NDC) as wv_pool:
            xt_sb = []
            for dc in range(NDC):
                t = xt_pool.tile([128, T], F32R, tag="xt", name=f"xt{dc}")
                nc.sync.dma_start(t[:], xT[dc * 128:(dc + 1) * 128, :])
                xt_sb.append(t)
            wv_sb = []
            for dc in range(NDC):
                t = wv_pool.tile([128, DL], F32R, tag="wv", name=f"wv{dc}")
                nc.sync.dma_start(t[:], wv[dc * 128:(dc + 1) * 128, :])
                wv_sb.append(t)

            # v: per t-chunk, [t(128), vcol(512)] = xT_chunk.T @ wv
            for tcn in range(NTC):
                ps_v = ps_tile("qkps", 2)
                for dc in range(NDC):
                    nc.tensor.matmul(
                        ps_v[:], xt_sb[dc][:, tcn * 128:(tcn + 1) * 128],
                        wv_sb[dc][:], start=(dc == 0), stop=(dc == NDC - 1))
                # scatter into v_sb pair-blocks (bf16 cast on copy)
                dst = v_sb[tcn][:].rearrange("p (e c) -> p e c", c=VPAIR)
                src = ps_v[:].rearrange("p (e c) -> p e c", c=128)
                nc.vector.tensor_copy(dst[:, :, 0:64], src[:, :, 0:64])
                nc.vector.tensor_copy(dst[:, :, 128:192], src[:, :, 64:128])
                nc.gpsimd.memset(dst[:, :, 64:65], 1.0)
                nc.gpsimd.memset(dst[:, :, 65:128], 0.0)

            # ------------- phase A1..A4: per e-chunk qk + attention -------------
            for ec in range(NEC):
                # load weight slices for this e-chunk
                wq_t, wk_t = [], []
                for dc in range(NDC):
                    t = wqk_pool.tile([128, 128], F32R, tag="wqk", name="wqkt")
                    nc.sync.dma_start(
                        t[:], wq[dc * 128:(dc + 1) * 128, ec * 128:(ec + 1) * 128])
                    wq_t.append(t)
                for dc in range(NDC):
                    t = wqk_pool.tile([128, 128], F32R, tag="wqk", name="wqkt")
                    nc.sync.dma_start(
                        t[:], wk[dc * 128:(dc + 1) * 128, ec * 128:(ec + 1) * 128])
                    wk_t.append(t)

                # qT / kT chunk [128, T] = W_chunk.T @ xT
                q_ec = qk_pool.tile([128, T], F32R, tag="qk")
                k_ec = qk_pool.tile([128, T], F32R, tag="qk")
                for (w_t, dst_t) in ((wq_t, q_ec), (wk_t, k_ec)):
                    for tbp in range(2):  # t-block pairs, 2 psum tiles each
                        pss = [ps_tile("qkps", 2) for _ in range(2)]
                        for dc in range(NDC):
                            for i in range(2):
                                tb = 2 * tbp + i
                                nc.tensor.matmul(
                                    pss[i], w_t[dc][:],
                                    xt_sb[dc][:, tb * 512:(tb + 1) * 512],
                                    start=(dc == 0), stop=(dc == NDC - 1))
                        for i in range(2):
                            tb = 2 * tbp + i
                            nc.vector.tensor_copy(
                                dst_t[:, tb * 512:(tb + 1) * 512], pss[i][:])

                # attention for the two heads of this chunk. i-blocks are
                # processed in pairs sharing a 2-bank score psum tile so one
                # exp call covers up to 1024 columns (ACT call overhead is
                # ~293ns, so fewer+bigger calls matter).
                for par in range(2):       # head parity within chunk
                    h = 2 * ec + par       # local head index
                    po = 64 * par          # partition offset in qk chunk
                    qh = q_ec[po:po + 64, :]
                    kh = k_ec[po:po + 64, :]
                    vt = v_sb_r = None
                    for ibp in range(2):   # i-block pair: (2*ibp, 2*ibp+1)
                        ibl, ibr = 2 * ibp, 2 * ibp + 1
                        utl = ps_tile("utps", 2)   # UT accum for left ib
                        utr = ps_tile("utps", 2)   # UT accum for right ib
                        for jt in range(4 * ibr + 4):
                            # col range of this jt within the 1024-wide pair
                            # (cols [0,512) = ibl, [512,1024) = ibr)
                            dl = (jt // 4 == ibl)   # diagonal for left ib
                            skip_l = (jt // 4 > ibl)
                            dr = (jt // 4 == ibr)   # diagonal for right ib
                            cl = 128 * (jt - 4 * ibl) if dl else 0
                            cr = 128 * (jt - 4 * ibr) if dr else 0
                            c0 = 512 + cr if skip_l else cl
                            st_ps = ps_pool.tile([128, 1024], F32, tag="stps",
                                                 name="ps_stps", bufs=2)
                            kh_j = kh[:, jt * 128:(jt + 1) * 128]
                            if not skip_l:
                                nc.tensor.matmul(
                                    st_ps[:, cl:512], kh_j,
                                    qh[:, ibl * 512 + cl:(ibl + 1) * 512],
                                    start=True, stop=True)
                            nc.tensor.matmul(
                                st_ps[:, 512 + cr:1024], kh_j,
                                qh[:, ibr * 512 + cr:(ibr + 1) * 512],
                                start=True, stop=True)
                            pt_t = pt_pool.tile([128, 1024], BF16, tag="pt")
                            nc.scalar.activation(
                                pt_t[:, c0:1024], st_ps[:, c0:1024], EXP,
                                scale=SCALE)
                            if dl:
                                nc.vector.tensor_mul(
                                    pt_t[:, cl:cl + 128], pt_t[:, cl:cl + 128],
                                    mk_sb[:])
                            if dr:
                                nc.vector.tensor_mul(
                                    pt_t[:, 512 + cr:512 + cr + 128],
                                    pt_t[:, 512 + cr:512 + cr + 128], mk_sb[:])
                            # PV: [v|1].T @ PT -> UT rows + denom row
                            vt = v_sb[jt][:].rearrange(
                                "p (e c) -> p e c", c=VPAIR)[:, ec, :]
                            if par == 0:
                                lhs = vt[:, 0:65]       # M=65 -> rows 0..64
                                rl, rr = slice(0, 65), slice(0, 65)
                            else:
                                # [one|junk63|v_odd]: denom row 0, v rows 64..127
                                lhs = vt[:, 64:192]     # M=128 -> rows 0..127
                                rl, rr = slice(0, 128), slice(0, 128)
                            if not skip_l:
                                nc.tensor.matmul(
                                    utl[rl, cl:512], lhs, pt_t[:, cl:512],
                                    start=(jt == 0), stop=(jt == 4 * ibl + 3),
                                    skip_group_check=True)
                            nc.tensor.matmul(
                                utr[rr, cr:512], lhs, pt_t[:, 512 + cr:1024],
                                start=(jt == 0), stop=(jt == 4 * ibr + 3),
                                skip_group_check=True)
                            for ib_d, ut_d in ((ibl, utl), (ibr, utr)):
                                if jt != 4 * ib_d + 3:
                                    continue
                                if par == 0:
                                    usrc, dsrc, r = ut_d[0:64, :], ut_d[64:65, :], 64
                                    udst = ut_sb[ec][0:64,
                                                     ib_d * 512:(ib_d + 1) * 512]
                                else:
                                    usrc, dsrc, r = ut_d[64:128, :], ut_d[0:1, :], 0
                                    udst = ut_sb[ec][64:128,
                                                     ib_d * 512:(ib_d + 1) * 512]
                                nc.vector.tensor_copy(udst, usrc)
                                # denom: same-partition copy to stage, DMA repack
                                stg = dns_pool.tile([128, 512], F32, tag="dns",
                                                    name="dnstg")
                                nc.vector.tensor_copy(stg[r:r + 1, :], dsrc)
                                nc.sync.dma_start(
                                    dn_sb[h:h + 1, ib_d * 512:(ib_d + 1) * 512],
                                    stg[r:r + 1, :])
        # xt/wv pools released here

        # ---------------- normalize: affinT = UT * (1/denom) ----------------
        rd = qk_pool.tile([HL, T], F32R, tag="qk")
        tmp = qk_pool.tile([HL, T], F32, tag="qk")
        r0 = qk_pool.tile([HL, T], F32, tag="qk")
        nc.vector.reciprocal(r0[:], dn_sb[:])
        # one Newton step: rd = r0 * (2 - dn*r0)
        nc.vector.tensor_mul(tmp[:], dn_sb[:], r0[:])
        nc.scalar.activation(tmp[:], tmp[:], COPY, scale=-1.0, bias=2.0)
        nc.vector.tensor_mul(rd[:], r0[:], tmp[:])

        for ec in range(NEC):
            for tb in range(NTB):
                ps_r = ps_tile("qkps", 2)
                nc.tensor.matmul(
                    ps_r[:], em_sb[:, ec * 128:(ec + 1) * 128],
                    rd[:, tb * 512:(tb + 1) * 512], start=True, stop=True)
                nc.vector.tensor_mul(
                    ut_sb[ec][:, tb * 512:(tb + 1) * 512],
                    ut_sb[ec][:, tb * 512:(tb + 1) * 512], ps_r[:])

        # ---------------- projection: out = affinT.T @ wp ----------------
        with tc.tile_pool(name="wp", bufs=NEC) as wp_pool, \
             tc.tile_pool(name="stage", bufs=3) as stage_pool:
            wp_sb = []
            for ecn in range(NEC):
                t = wp_pool.tile([128, D], F32R, tag="wp", name=f"wpt{ecn}")
                nc.sync.dma_start(t[:], wp[ecn * 128:(ecn + 1) * 128, :])
                wp_sb.append(t)
            for tcn in range(NTC):
                for ob in range(2):
                    ps_p = ps_tile("qkps", 2)
                    for ecn in range(NEC):
                        nc.tensor.matmul(
                            ps_p[:], ut_sb[ecn][:, tcn * 128:(tcn + 1) * 128],
                            wp_sb[ecn][:, ob * 512:(ob + 1) * 512],
                            start=(ecn == 0), stop=(ecn == NEC - 1))
                    st = stage_pool.tile([128, 512], F32, tag="st")
                    nc.vector.tensor_copy(st[:], ps_p[:])
                    nc.sync.dma_start(
                        out[tcn * 128:(tcn + 1) * 128, ob * 512:(ob + 1) * 512],
                        st[:])


_NC_CACHE = None


def _get_nc():
    global _NC_CACHE
    if _NC_CACHE is None:
        _NC_CACHE = _build()
    return _NC_CACHE


def make_in_maps(embds, W_qkv, W_proj):
    embds = np.asarray(embds, dtype=np.float32)
    W_qkv = np.asarray(W_qkv, dtype=np.float32)
    W_proj = np.asarray(W_proj, dtype=np.float32)

    mask_np = np.triu(np.ones((128, 128))).astype(ml_dtypes.bfloat16)
    emat_np = np.kron(np.eye(HL), np.ones((1, Dh))).astype(np.float32)

    in_maps = []
    for c in range(NCORES):
        b, hg = c // 2, c % 2
        sl = slice(hg * DL, (hg + 1) * DL)
        in_maps.append({
            "xT": np.ascontiguousarray(embds[b].T),
            "wk": np.ascontiguousarray(W_qkv[:, 0 * D:1 * D][:, sl]),
            "wq": np.ascontiguousarray(W_qkv[:, 1 * D:2 * D][:, sl]),
            "wv": np.ascontiguousarray(W_qkv[:, 2 * D:3 * D][:, sl]),
            "wp": np.ascontiguousarray(W_proj[sl, :]),
            "mask": mask_np,
            "emat": emat_np,
        })
    return in_maps


def gather_out(outs, b_proj):
    b_proj = np.asarray(b_proj, dtype=np.float32)
    full = np.empty((B, T, D), dtype=np.float32)
    for b in range(B):
        full[b] = outs[2 * b] + outs[2 * b + 1] + b_proj[None, :]
    return full


def kernel(embds, W_qkv, W_proj, b_proj):
    in_maps = make_in_maps(embds, W_qkv, W_proj)
    nc = _get_nc()
    res = run_bass_kernel_spmd(nc, in_maps, list(range(NCORES)))
    return gather_out([r["out"] for r in res.results], b_proj)
